# revision 17
# baseline (speedup 1.0000x reference)
"""BertSelfAttention (ALiBi-style additive bias) on 8 TRN2 NeuronCores.

Problem: B=4, S=1024, D=1024, H=16 heads (HD=64), fp32.
  qkv = hidden @ Wqkv_w.T + Wqkv_b
  scores = q @ k.T / sqrt(64) + bias ;  probs = softmax(scores) ; out = probs @ v

Sharding: 8 cores = 4 batches x 2 head-groups. Core c handles batch c//2 and
heads [ (c%2)*8, (c%2)*8+8 ).  Per-core shards are prepared host-side in the
layouts the TensorEngine wants (contraction dim on partitions), so every
device DMA is a contiguous, full-rate read:
  hT  [D, S]        = hidden[b].T
  wT  [D, 1536]     = Wqkv rows for this core's (q|k|v) heads, transposed
  wb  [1, 1536]     = matching bias slice
  bT  [8, S, S]     = bias[b, h].T per head  (scores are computed transposed)
Device computes, per head, scoresT[k, q] = kT.T @ qT (+ biasT via an
identity-matmul accumulated into the same PSUM tile), exp (no max-subtraction:
scores+bias <= ~10 so fp32 exp cannot overflow; large-negative bias cleanly
underflows to 0), then outT[d, q] = [v | 1].T @ expT, which also yields the
softmax denominator as row 64.  Normalization happens on-chip (DVE reciprocal
+ GpSimd partition-broadcast).  The host only re-transposes the per-core
[512, S] result into the final (B, S, D) tensor.
"""

import numpy as np

import concourse.bacc as bacc
import concourse.bass as bass
import concourse.mybir as mybir
from concourse.tile import TileContext

B, S, D = 4, 1024, 1024
H = 16
HD = 64  # head dim
N_CORES = 8
HPC = 8  # heads per core
OC = 3 * HPC * HD  # 1536 fused-qkv output rows per core
F32 = mybir.dt.float32
F32R = mybir.dt.float32r

KC = S // 128  # 8 key-token chunks of 128
TC_ = S // 128  # 8 token chunks of 128
DC = D // 128  # 8 contraction chunks of 128


def build_bass() -> bass.Bass:
    nc = bacc.Bacc()

    hw = nc.declare_dram_parameter("hw", [D, S + OC], F32R, isOutput=False)
    wb = nc.declare_dram_parameter("wb", [1, 2 * OC], F32R, isOutput=False)
    bT = nc.declare_dram_parameter("bT", [HPC, S, S], F32R, isOutput=False)
    idm = nc.declare_dram_parameter("idm", [128, 128], F32R, isOutput=False)
    oT = nc.declare_dram_parameter("oT", [HPC * HD, S], F32, isOutput=True)

    with TileContext(nc) as tc:
        with (
            tc.tile_pool(name="const", bufs=1) as constp,
            tc.tile_pool(name="weights", bufs=1) as wp,
            tc.tile_pool(name="qk", bufs=1) as qkp,
            tc.tile_pool(name="vex", bufs=1) as vp,
            tc.tile_pool(name="bias", bufs=4) as btp,
            tc.tile_pool(name="exp", bufs=3) as ep,
            tc.tile_pool(name="outs", bufs=2) as op_,
            tc.tile_pool(name="ps_mm", bufs=2, space="PSUM") as ps_mm,
            tc.tile_pool(name="ps_v", bufs=2, space="PSUM") as ps_v,
            tc.tile_pool(name="ps_o", bufs=1, space="PSUM") as ps_o,
        ):
            # --- constants -------------------------------------------------
            ident = constp.tile([128, 128], F32R)
            nc.sync.dma_start(out=ident[:], in_=idm[:])
            # wb row: [fused qkv bias slice (OC) | all-ones (OC)] on one
            # partition, so bias matmuls and ones-operand matmuls share one
            # DMA semaphore
            wb_sb = constp.tile([1, 2 * OC], F32R)
            nc.sync.dma_start(out=wb_sb[:], in_=wb[:])
            ones = wb_sb[:, OC : 2 * OC]

            # --- stage inputs ---------------------------------------------
            # one DMA per 128-row chunk carrying both hidden^T and W^T, so
            # each first consumer matmul waits on a single DMA semaphore
            hT_sb = []
            wT_sb = []
            for c in range(DC):
                hwt = wp.tile([128, S + OC], F32R, tag=f"hw{c}", name=f"hw{c}")
                nc.sync.dma_start(out=hwt[:], in_=hw[c * 128 : (c + 1) * 128, :])
                hT_sb.append(hwt[:, 0:S])
                wT_sb.append(hwt[:, S : S + OC])

            # --- phase 1: fused QKV projection -----------------------------
            # qkT_sb[j][p, t]: j in 0..3 -> q rows (pre-scaled by 1/8),
            #                  j in 4..7 -> k rows. Row (j%4)*128+p = oc index.
            qk_sb = [qkp.tile([128, S], F32R, tag=f"qk{j}", name=f"qk{j}") for j in range(8)]
            # v_sb[t][p, h, 0:64] = v head h, token t*128+p; [.., 64] = 1.0
            v_sb = [vp.tile([128, HPC, HD + 1], F32R, tag=f"vx{t}", name=f"v{t}") for t in range(TC_)]

            def qk_block(j):
                # psum[p, t] = qkv^T rows j*128..j*128+128 (oc on partitions)
                ps = ps_mm.tile([128, S], F32, tag="mm")
                for c in range(DC):
                    lw = wT_sb[c][:, j * 128 : (j + 1) * 128]
                    for half in range(2):
                        nc.tensor.matmul(
                            ps[:, half * 512 : (half + 1) * 512],
                            lw,
                            hT_sb[c][:, half * 512 : (half + 1) * 512],
                            start=(c == 0),
                            stop=False,
                        )
                for half in range(2):
                    nc.tensor.matmul(
                        ps[:, half * 512 : (half + 1) * 512],
                        wb_sb[:, j * 128 : (j + 1) * 128],
                        ones[:, half * 512 : (half + 1) * 512],
                        start=False,
                        stop=True,
                    )
                # copy to SBUF; fold the 1/sqrt(HD) score scale into q rows
                scale = 0.125 if j < 4 else 1.0
                nc.scalar.activation(
                    qk_sb[j][:], ps[:], mybir.ActivationFunctionType.Copy, scale=scale
                )

            def v_block(t):
                ps = ps_v.tile([128, HPC * HD], F32, tag="v")
                for c in range(DC):
                    nc.tensor.matmul(
                        ps[:],
                        hT_sb[c][:, t * 128 : (t + 1) * 128],
                        wT_sb[c][:, 2 * HPC * HD : 3 * HPC * HD],
                        start=(c == 0),
                        stop=False,
                    )
                nc.tensor.matmul(
                    ps[:],
                    ones[:, t * 128 : (t + 1) * 128],
                    wb_sb[:, 2 * HPC * HD : 3 * HPC * HD],
                    start=False,
                    stop=True,
                )
                nc.scalar.activation(
                    v_sb[t][:, :, 0:HD],
                    ps[:].rearrange("p (h d) -> p h d", h=HPC),
                    mybir.ActivationFunctionType.Copy,
                )
                nc.scalar.activation(
                    v_sb[t][:, :, HD : HD + 1],
                    v_sb[t][:, :, 0:1],
                    mybir.ActivationFunctionType.Identity,
                    scale=0.0,
                    bias=1.0,
                )

            # q/k for heads 0,1 first, then v (AV needs all of it), then rest
            qk_block(0)
            qk_block(4)
            for t in range(TC_):
                v_block(t)
            for j in (1, 5, 2, 6, 3, 7):
                qk_block(j)

            # --- phase 2: attention ----------------------------------------
            for h in range(HPC):
                j, po = h // 2, (h % 2) * 64
                qT = qk_sb[j][po : po + 64, :]  # [64, S] (already /8)
                kT = qk_sb[4 + j][po : po + 64, :]  # [64, S]
                po_ps = ps_o.tile([128, S], F32, tag="o")  # rows 0..64 used
                for kc in range(KC):
                    bt = btp.tile([128, S], F32R, tag="bt")
                    nc.sync.dma_start(
                        out=bt[:], in_=bT[h, kc * 128 : (kc + 1) * 128, :]
                    )
                    ps = ps_mm.tile([128, S], F32, tag="mm")
                    # scoresT[k, q] = k @ q.T   (contraction over head dim)
                    for half in range(2):
                        nc.tensor.matmul(
                            ps[:, half * 512 : (half + 1) * 512],
                            kT[:, kc * 128 : (kc + 1) * 128],
                            qT[:, half * 512 : (half + 1) * 512],
                            start=True,
                            stop=False,
                        )
                    # += biasT via identity matmul (I.T @ bt = bt)
                    for half in range(2):
                        nc.tensor.matmul(
                            ps[:, half * 512 : (half + 1) * 512],
                            ident[:],
                            bt[:, half * 512 : (half + 1) * 512],
                            start=False,
                            stop=True,
                        )
                    et = ep.tile([128, S], F32R, tag="et")
                    nc.scalar.activation(
                        et[:], ps[:], mybir.ActivationFunctionType.Exp
                    )
                    # outT[d, q] += v_ext.T @ expT ; row 64 accumulates sum(exp)
                    for half in range(2):
                        nc.tensor.matmul(
                            po_ps[0 : HD + 1, half * 512 : (half + 1) * 512],
                            v_sb[kc][:, h, :],
                            et[:, half * 512 : (half + 1) * 512],
                            start=(kc == 0),
                            stop=(kc == KC - 1),
                        )
                # normalize: out[d, q] * (1 / sum[q]).  Copy the unnormalized
                # AV result to SBUF, broadcast 1/sum over PSUM rows 0..63 via
                # a K=1 matmul (base partition 0), then multiply.
                av = op_.tile([HD, S], F32, tag="av")
                nc.vector.tensor_copy(av[:], po_ps[0:HD, :])
                recip = op_.tile([1, S], F32R, tag="recip")
                with nc.allow_low_precision(reason="fp32r recip feeds matmul"):
                    nc.vector.reciprocal(recip[:], po_ps[HD : HD + 1, :])
                for half in range(2):
                    nc.tensor.matmul(
                        po_ps[0:HD, half * 512 : (half + 1) * 512],
                        ones[:, 0:HD],
                        recip[:, half * 512 : (half + 1) * 512],
                        start=True,
                        stop=True,
                    )
                ot = op_.tile([HD, S], F32, tag="ot")
                nc.vector.tensor_tensor(
                    ot[:], po_ps[0:HD, :], av[:], op=mybir.AluOpType.mult
                )
                nc.sync.dma_start(out=oT[h * HD : (h + 1) * HD, :], in_=ot[:])

    # Bacc defers register allocation to its compile() pass, which only runs
    # in finalize(); run_bass_via_pjrt ships the BIR as-is, so finalize here.
    nc.finalize()
    return nc


def shard_inputs(hidden_states, bias, Wqkv_w, Wqkv_b):
    """Slice + lay out the full inputs into 8 per-core input maps."""
    hidden_states = np.asarray(hidden_states, dtype=np.float32)
    bias = np.asarray(bias, dtype=np.float32)
    Wqkv_w = np.asarray(Wqkv_w, dtype=np.float32)
    Wqkv_b = np.asarray(Wqkv_b, dtype=np.float32)

    in_maps = []
    eye = np.eye(128, dtype=np.float32)
    for c in range(N_CORES):
        b, hs = c // 2, (c % 2) * HPC
        rows = np.concatenate(
            [np.arange(g * D + hs * HD, g * D + (hs + HPC) * HD) for g in range(3)]
        )
        wb2 = np.ones((1, 2 * OC), dtype=np.float32)
        wb2[0, :OC] = Wqkv_b[rows]
        in_maps.append(
            {
                "hw": np.ascontiguousarray(
                    np.concatenate([hidden_states[b].T, Wqkv_w[rows].T], axis=1)
                ),
                "wb": wb2,
                "bT": np.ascontiguousarray(
                    bias[b, hs : hs + HPC].transpose(0, 2, 1)
                ),
                "idm": eye,
            }
        )
    return in_maps


_CACHED_NC = None


def kernel(hidden_states, bias, Wqkv_w, Wqkv_b):
    from concourse.bass_utils import run_bass_kernel_spmd

    global _CACHED_NC
    if _CACHED_NC is None:
        _CACHED_NC = build_bass()
    in_maps = shard_inputs(hidden_states, bias, Wqkv_w, Wqkv_b)
    res = run_bass_kernel_spmd(_CACHED_NC, in_maps, core_ids=list(range(N_CORES)))
    out = np.empty((B, S, D), dtype=np.float32)
    for c in range(N_CORES):
        b, hs = c // 2, (c % 2) * HPC
        out[b, :, hs * HD : (hs + HPC) * HD] = res.results[c]["oT"].T
    return out


# revision 19
# speedup vs baseline: 1.5508x; 1.5508x over previous
"""BertSelfAttention (ALiBi-style additive bias) on 8 TRN2 NeuronCores.

Problem: B=4, S=1024, D=1024, H=16 heads (HD=64), fp32.
  qkv = hidden @ Wqkv_w.T + Wqkv_b
  scores = q @ k.T / sqrt(64) + bias ;  probs = softmax(scores) ; out = probs @ v

Sharding: 8 cores = 4 batches x 2 head-groups. Core c handles batch c//2 and
heads [ (c%2)*8, (c%2)*8+8 ).  Per-core shards are prepared host-side in the
layouts the TensorEngine wants (contraction dim on partitions) and cast to
bf16 (TensorE runs bf16 at full rate with fast weight loads; accumulation
stays fp32 in PSUM), so every device DMA is a contiguous, full-rate read:
  hw  [D, S+1536]  = [hidden[b].T | Wqkv rows for this core, transposed]
  wb  [1, 2*1536]  = [fused qkv bias slice | all-ones row]
  bT  [8, S, S]    = bias[b, h].T per head  (scores are computed transposed)
  idm [128, 128]   = identity (for the bias-add-by-matmul)
Device, per head: scoresT[k, q] = kT.T @ qT + biasT (identity-matmul
accumulated into the same PSUM tile), exp on ScalarE (no max-subtraction:
scores+bias <= ~10 so fp32 exp cannot overflow; large-negative ALiBi bias
cleanly underflows to 0), then outT[d, q] = [v | 1].T @ expT per 512-column
half, which also yields the softmax denominator in row 64.  Normalization =
broadcast the denominator over rows with a K=1 matmul, then fp32 DVE divide.
The host only re-transposes the per-core [512, S] result into (B, S, D).
"""

import numpy as np

import concourse.bacc as bacc
import concourse.bass as bass
import concourse.mybir as mybir
from concourse.tile import TileContext

B, S, D = 4, 1024, 1024
H = 16
HD = 64  # head dim
N_CORES = 8
HPC = 8  # heads per core
OC = 3 * HPC * HD  # 1536 fused-qkv output rows per core
F32 = mybir.dt.float32
BF16 = mybir.dt.bfloat16

KC = S // 128  # 8 key-token chunks of 128
TC_ = S // 128  # 8 token chunks of 128
DC = D // 128  # 8 contraction chunks of 128


def build_bass() -> bass.Bass:
    nc = bacc.Bacc()

    hw = nc.declare_dram_parameter("hw", [D, S + OC], BF16, isOutput=False)
    wb = nc.declare_dram_parameter("wb", [1, 2 * OC], BF16, isOutput=False)
    bT = nc.declare_dram_parameter("bT", [HPC, S, S], BF16, isOutput=False)
    idm = nc.declare_dram_parameter("idm", [128, 128], BF16, isOutput=False)
    oT = nc.declare_dram_parameter("oT", [HPC * HD, S], F32, isOutput=True)

    with TileContext(nc) as tc:
        with (
            tc.tile_pool(name="const", bufs=1) as constp,
            tc.tile_pool(name="weights", bufs=1) as wp,
            tc.tile_pool(name="qk", bufs=1) as qkp,
            tc.tile_pool(name="vex", bufs=1) as vp,
            tc.tile_pool(name="bias", bufs=4) as btp,
            tc.tile_pool(name="exp", bufs=3) as ep,
            tc.tile_pool(name="outs", bufs=4) as op_,
            tc.tile_pool(name="ps_mm", bufs=2, space="PSUM") as ps_mm,
            tc.tile_pool(name="ps_sm", bufs=4, space="PSUM") as ps_sm,
        ):
            # --- constants -------------------------------------------------
            ident = constp.tile([128, 128], BF16)
            nc.sync.dma_start(out=ident[:], in_=idm[:])
            # wb row: [fused qkv bias slice (OC) | all-ones (OC)] on one
            # partition, so bias matmuls and ones-operand matmuls share one
            # DMA semaphore
            wb_sb = constp.tile([1, 2 * OC], BF16)
            nc.sync.dma_start(out=wb_sb[:], in_=wb[:])
            ones = wb_sb[:, OC : 2 * OC]

            # --- stage inputs ---------------------------------------------
            # one DMA per 128-row chunk carrying both hidden^T and W^T, so
            # each first consumer matmul waits on a single DMA semaphore
            hT_sb = []
            wT_sb = []
            for c in range(DC):
                hwt = wp.tile([128, S + OC], BF16, tag=f"hw{c}", name=f"hw{c}")
                nc.sync.dma_start(out=hwt[:], in_=hw[c * 128 : (c + 1) * 128, :])
                hT_sb.append(hwt[:, 0:S])
                wT_sb.append(hwt[:, S : S + OC])

            # --- phase 1: fused QKV projection -----------------------------
            # qkT_sb[j][p, t]: j in 0..3 -> q rows (pre-scaled by 1/8),
            #                  j in 4..7 -> k rows. Row (j%4)*128+p = oc index.
            qk_sb = [
                qkp.tile([128, S], BF16, tag=f"qk{j}", name=f"qk{j}")
                for j in range(8)
            ]
            # v_sb[t][p, h, 0:64] = v head h, token t*128+p; [.., 64] = 1.0
            v_sb = [
                vp.tile([128, HPC, HD + 1], BF16, tag=f"vx{t}", name=f"v{t}")
                for t in range(TC_)
            ]

            def qk_block(j):
                # psum[p, t] = qkv^T rows j*128..j*128+128 (oc on partitions)
                ps = ps_mm.tile([128, S], F32, tag="mm")
                for c in range(DC):
                    lw = wT_sb[c][:, j * 128 : (j + 1) * 128]
                    for half in range(2):
                        nc.tensor.matmul(
                            ps[:, half * 512 : (half + 1) * 512],
                            lw,
                            hT_sb[c][:, half * 512 : (half + 1) * 512],
                            start=(c == 0),
                            stop=False,
                        )
                for half in range(2):
                    nc.tensor.matmul(
                        ps[:, half * 512 : (half + 1) * 512],
                        wb_sb[:, j * 128 : (j + 1) * 128],
                        ones[:, half * 512 : (half + 1) * 512],
                        start=False,
                        stop=True,
                    )
                # copy to SBUF; fold the 1/sqrt(HD) score scale into q rows
                scale = 0.125 if j < 4 else 1.0
                nc.scalar.activation(
                    qk_sb[j][:], ps[:], mybir.ActivationFunctionType.Copy, scale=scale
                )

            def v_block(t):
                ps = ps_sm.tile([128, HPC * HD], F32, tag="sm")
                for c in range(DC):
                    nc.tensor.matmul(
                        ps[:],
                        hT_sb[c][:, t * 128 : (t + 1) * 128],
                        wT_sb[c][:, 2 * HPC * HD : 3 * HPC * HD],
                        start=(c == 0),
                        stop=False,
                    )
                nc.tensor.matmul(
                    ps[:],
                    ones[:, t * 128 : (t + 1) * 128],
                    wb_sb[:, 2 * HPC * HD : 3 * HPC * HD],
                    start=False,
                    stop=True,
                )
                nc.scalar.activation(
                    v_sb[t][:, :, 0:HD],
                    ps[:].rearrange("p (h d) -> p h d", h=HPC),
                    mybir.ActivationFunctionType.Copy,
                )
                nc.scalar.activation(
                    v_sb[t][:, :, HD : HD + 1],
                    v_sb[t][:, :, 0:1],
                    mybir.ActivationFunctionType.Identity,
                    scale=0.0,
                    bias=1.0,
                )

            # q/k for heads 0,1 first, then v (AV needs all of it), then rest
            qk_block(0)
            qk_block(4)
            for t in range(TC_):
                v_block(t)
            for j in (1, 5, 2, 6, 3, 7):
                qk_block(j)

            # --- phase 2: attention ----------------------------------------
            for h in range(HPC):
                j, po = h // 2, (h % 2) * 64
                qT = qk_sb[j][po : po + 64, :]  # [64, S] (already /8)
                kT = qk_sb[4 + j][po : po + 64, :]  # [64, S]
                # two 1-bank [65, 512] output tiles (rows 0..63 = outT,
                # row 64 = sum of exp); separate tiles keep each matmul
                # within one PSUM bank and let heads double-buffer.
                pos = [
                    ps_sm.tile([HD + 1, 512], F32, tag="sm", name=f"po{h}_{i}")
                    for i in range(2)
                ]
                for kc in range(KC):
                    bt = btp.tile([128, S], BF16, tag="bt")
                    nc.sync.dma_start(
                        out=bt[:], in_=bT[h, kc * 128 : (kc + 1) * 128, :]
                    )
                    ps = ps_mm.tile([128, S], F32, tag="mm")
                    # scoresT[k, q] = k @ q.T   (contraction over head dim)
                    for half in range(2):
                        nc.tensor.matmul(
                            ps[:, half * 512 : (half + 1) * 512],
                            kT[:, kc * 128 : (kc + 1) * 128],
                            qT[:, half * 512 : (half + 1) * 512],
                            start=True,
                            stop=False,
                        )
                    # += biasT via identity matmul (I.T @ bt = bt)
                    for half in range(2):
                        nc.tensor.matmul(
                            ps[:, half * 512 : (half + 1) * 512],
                            ident[:],
                            bt[:, half * 512 : (half + 1) * 512],
                            start=False,
                            stop=True,
                        )
                    et = ep.tile([128, S], BF16, tag="et")
                    nc.scalar.activation(
                        et[:], ps[:], mybir.ActivationFunctionType.Exp
                    )
                    # outT[d, q] += v_ext.T @ expT ; row 64 accumulates sum(exp)
                    for half in range(2):
                        nc.tensor.matmul(
                            pos[half][:],
                            v_sb[kc][:, h, :],
                            et[:, half * 512 : (half + 1) * 512],
                            start=(kc == 0),
                            stop=(kc == KC - 1),
                        )
                # normalize: out[d, q] * (1/sum[q]), per 512-column half.
                # 1/sum via reciprocal_approx_fast (18 bits, sums are benign),
                # broadcast over PSUM rows 0..63 with a K=1 matmul, multiply.
                for half in range(2):
                    p = pos[half]
                    av = op_.tile([HD, 512], F32, tag="av")
                    nc.vector.tensor_copy(av[:], p[0:HD, :])
                    smf = op_.tile([1, 512], F32, tag="smf")
                    nc.vector.tensor_copy(smf[:], p[HD : HD + 1, :])
                    rcf = op_.tile([1, 512], F32, tag="rcf")
                    nc.vector.reciprocal_approx_fast(rcf[:], smf[:])
                    rcb = op_.tile([1, 512], BF16, tag="rcb")
                    nc.vector.tensor_copy(rcb[:], rcf[:])
                    nc.tensor.matmul(
                        p[0:HD, :], ones[:, 0:HD], rcb[:], start=True, stop=True
                    )
                    ot = op_.tile([HD, 512], F32, tag="ot")
                    nc.vector.tensor_tensor(
                        ot[:], av[:], p[0:HD, :], op=mybir.AluOpType.mult
                    )
                    nc.sync.dma_start(
                        out=oT[h * HD : (h + 1) * HD, half * 512 : (half + 1) * 512],
                        in_=ot[:],
                    )

    # Bacc defers register allocation to its compile() pass, which only runs
    # in finalize(); run_bass_via_pjrt ships the BIR as-is, so finalize here.
    nc.finalize()
    return nc


def shard_inputs(hidden_states, bias, Wqkv_w, Wqkv_b):
    """Slice + lay out the full inputs into 8 per-core input maps."""
    import ml_dtypes

    bf16 = ml_dtypes.bfloat16
    hidden_states = np.asarray(hidden_states, dtype=np.float32)
    bias = np.asarray(bias, dtype=np.float32)
    Wqkv_w = np.asarray(Wqkv_w, dtype=np.float32)
    Wqkv_b = np.asarray(Wqkv_b, dtype=np.float32)

    in_maps = []
    eye = np.eye(128, dtype=bf16)
    for c in range(N_CORES):
        b, hs = c // 2, (c % 2) * HPC
        rows = np.concatenate(
            [np.arange(g * D + hs * HD, g * D + (hs + HPC) * HD) for g in range(3)]
        )
        wb2 = np.ones((1, 2 * OC), dtype=bf16)
        wb2[0, :OC] = Wqkv_b[rows].astype(bf16)
        in_maps.append(
            {
                "hw": np.concatenate(
                    [hidden_states[b].T, Wqkv_w[rows].T], axis=1
                ).astype(bf16),
                "wb": wb2,
                "bT": np.ascontiguousarray(
                    bias[b, hs : hs + HPC].transpose(0, 2, 1)
                ).astype(bf16),
                "idm": eye,
            }
        )
    return in_maps


_CACHED_NC = None


def kernel(hidden_states, bias, Wqkv_w, Wqkv_b):
    from concourse.bass_utils import run_bass_kernel_spmd

    global _CACHED_NC
    if _CACHED_NC is None:
        _CACHED_NC = build_bass()
    in_maps = shard_inputs(hidden_states, bias, Wqkv_w, Wqkv_b)
    res = run_bass_kernel_spmd(_CACHED_NC, in_maps, core_ids=list(range(N_CORES)))
    out = np.empty((B, S, D), dtype=np.float32)
    for c in range(N_CORES):
        b, hs = c // 2, (c % 2) * HPC
        out[b, :, hs * HD : (hs + HPC) * HD] = res.results[c]["oT"].T
    return out
